# revision 1
# baseline (speedup 1.0000x reference)
"""FP8-palettized linear kernel for 8x TRN2 NeuronCores.

Computes: out[b,s,o] = sum_d input[b,s,d] * lookup_table[weight[o,d]] + bias[o]
with input [4,2048,4096] f32, weight [4096,4096] int32 (palette ids < 256),
lookup_table [256] f32, bias [4096] f32.

Strategy (column-parallel, per sharding hint):
  - Each core owns a 512-wide slice of out_features. Full input replicated.
  - Host prep is value-preserving layout only: X is transposed to XT [d, s]
    (contraction on partitions for the PE), the int32 palette indices are
    permuted into the GPSIMD 16-partition "wrapped" order and fed as an
    int16 byte-view (values < 256 live in the low half of each int32).
  - On device: GPSIMD ap_gather dequantizes W^T k-tiles from a
    partition-broadcast LUT; a 16-phase SBUF->SBUF DMA compacts the
    16x-redundant gather output into dense [128 d, 512 o] fp32r tiles that
    stay resident in SBUF. TensorE then runs X^T-slab @ W^T with PSUM
    accumulation over the 32 k-tiles, DVE adds the bias, and results DMA out.
"""

import contextlib
import os

import numpy as np

import concourse.bacc as bacc
import concourse.mybir as mybir
import concourse.tile as tile
from concourse.bass_utils import run_bass_kernel_spmd

P = 128
N_CORES = 8

# Full-problem dims (hardcoded per harness contract).
BATCH, SEQ, D_IN, D_OUT, PALETTE = 4, 2048, 4096, 4096, 256
M_FULL = BATCH * SEQ  # 8192

# Matmul input dtype: float32r streams at 1 cycle/row on the PE for free
# dim >= 256 (fp32 costs 4). Overridable for precision/perf experiments.
MM_DTYPE = {
    "f32": mybir.dt.float32,
    "f32r": mybir.dt.float32r,
    "bf16": mybir.dt.bfloat16,
}[os.environ.get("PAL_MM_DTYPE", "f32r")]


def wrap_indices(w_shard: np.ndarray, n_ktiles: int) -> np.ndarray:
    """Permute a [OSH, K] int32 index shard into the ap_gather wrapped layout.

    Device contract (per k-tile kt, per 16-partition group g):
      unwrapped_g[i] = idx[16g + i%16, i//16] for i in [0, 16*OSH)
      after compaction row 16g+r takes segment i in [r*OSH, (r+1)*OSH):
      need unwrapped_g[r*OSH + j] = w_shard[j, kt*128 + 16g + r].
    Returns [128, n_ktiles*OSH] int32.
    """
    osh, k = w_shard.shape
    assert k == n_ktiles * P
    w4 = w_shard.reshape(osh, n_ktiles, 8, 16)  # [o][kt][g][r]
    u = w4.transpose(1, 2, 3, 0)  # [kt][g][r][o]
    wr = u.reshape(n_ktiles, 8, osh * 16 // 16, 16 * 16 // 16)  # placeholder
    wr = u.reshape(n_ktiles, 8, 16 * osh).reshape(n_ktiles, 8, osh, 16)
    wr = wr.transpose(1, 3, 0, 2)  # [g][p16][kt][s]
    return np.ascontiguousarray(wr.reshape(P, n_ktiles * osh))


def build_program(nc, *, m, k, osh, reps=1):
    """Emit the per-core Tile program. m: rows of X (mult of 128), k: d dim
    (mult of 128), osh: out-features per core (512). reps>1 wraps the body
    in a hardware loop (for benchmarking: amortizes dispatch overhead)."""
    n_kt = k // P
    n_mt = m // P
    f_red = 16 * osh  # gather output free size (per-partition)

    xt = nc.dram_tensor("xt", [k, m], MM_DTYPE, kind="ExternalInput")
    # int32 indices fed as int16 byte-view: value at even positions.
    widx = nc.dram_tensor("widx", [P, n_kt * osh * 2], mybir.dt.int16,
                          kind="ExternalInput")
    # Expanded per-partition table: row p holds LUT at slot (p%16)*256, zeros
    # elsewhere. Gathering with seg*256+idx leaves only partition p's own
    # segment nonzero, so a strided sum over the 16 segments compacts the
    # 16x-redundant gather output with plain (legal) free-dim APs.
    lutx = nc.dram_tensor("lutx", [P, 16 * PALETTE], mybir.dt.float32,
                          kind="ExternalInput")
    # ramp[p, s] = 256 * segment of wrapped slot (p%16, s)
    ramp = nc.dram_tensor("ramp", [P, osh], mybir.dt.int16,
                          kind="ExternalInput")
    bias = nc.dram_tensor("bias", [1, osh], mybir.dt.float32,
                          kind="ExternalInput")
    out = nc.dram_tensor("out", [m, osh], mybir.dt.float32,
                         kind="ExternalOutput")

    with tile.TileContext(nc) as tc:
        with (
            tc.tile_pool(name="const", bufs=1) as const_pool,
            tc.tile_pool(name="idx", bufs=2) as idx_pool,
            tc.tile_pool(name="idxc", bufs=2) as idxc_pool,
            tc.tile_pool(name="red", bufs=1) as red_pool,
            tc.tile_pool(name="wt", bufs=1) as wt_pool,
            tc.tile_pool(name="xs", bufs=3) as x_pool,
            tc.tile_pool(name="psum", bufs=2, space="PSUM") as psum_pool,
            tc.tile_pool(name="osb", bufs=2) as osb_pool,
            (tc.For_i(0, reps, 1) if reps > 1
             else contextlib.nullcontext()),
        ):
            # --- constants ---
            lutx_sb = const_pool.tile([P, 16 * PALETTE], mybir.dt.float32,
                                      tag="lutx")
            nc.sync.dma_start(lutx_sb[:], lutx[:])
            ramp_sb = const_pool.tile([P, osh], mybir.dt.int16, tag="ramp")
            nc.sync.dma_start(ramp_sb[:], ramp[:])

            bias_row = const_pool.tile([1, osh], mybir.dt.float32, tag="brow")
            nc.sync.dma_start(bias_row[:], bias[:])
            bias_sb = const_pool.tile([P, osh], mybir.dt.float32, tag="bsb")
            nc.gpsimd.partition_broadcast(bias_sb[:], bias_row[:])

            # --- dequant: one W^T tile [128 d, osh o] per k-tile ---
            # `red` is allocated once and reused so its slot is never handed
            # to another pool mid-flight.
            red = red_pool.tile([P, f_red], mybir.dt.float32)
            wt_tiles = []
            for kt in range(n_kt):
                idxr = idx_pool.tile([P, 2 * osh], mybir.dt.int16)
                nc.sync.dma_start(
                    idxr[:], widx[:, kt * 2 * osh:(kt + 1) * 2 * osh])
                idxc = idxc_pool.tile([P, osh], mybir.dt.int16)
                # int16 view of int32 values sits at even slots; add the
                # 256*segment ramp while compacting to contiguous int16.
                nc.vector.tensor_tensor(
                    idxc[:],
                    idxr.rearrange("p (s two) -> p s two", two=2)[:, :, 0],
                    ramp_sb[:],
                    op=mybir.AluOpType.add)

                nc.gpsimd.ap_gather(
                    red[:], lutx_sb[:], idxc[:],
                    channels=P, num_elems=16 * PALETTE, d=1, num_idxs=f_red)

                # sum over the 16 segments (only partition's own is nonzero)
                wt = wt_pool.tile([P, osh], MM_DTYPE,
                                  tag=f"wt{kt:02d}")
                # exact: 15 zeros + the partition's own segment value
                with nc.allow_low_precision(reason="sum of one value + zeros"):
                    nc.vector.tensor_reduce(
                        wt[:],
                        red.rearrange("p (r j) -> p j r", r=16),
                        axis=mybir.AxisListType.X,
                        op=mybir.AluOpType.add)
                wt_tiles.append(wt)

            # --- matmul: out[m-tile, :] = XT-slab^T @ W^T (+bias) ---
            for mt in range(n_mt):
                xslab = x_pool.tile([P, k], MM_DTYPE)
                nc.sync.dma_start(
                    xslab.rearrange("p (kt j) -> p kt j", kt=n_kt),
                    xt[:, mt * P:(mt + 1) * P]
                    .rearrange("(kt p) j -> p kt j", p=P))
                psum = psum_pool.tile([P, osh], mybir.dt.float32)
                for kt in range(n_kt):
                    nc.tensor.matmul(
                        psum[:],
                        lhsT=xslab[:, kt * P:(kt + 1) * P],
                        rhs=wt_tiles[kt][:],
                        start=(kt == 0),
                        stop=(kt == n_kt - 1))
                osb = osb_pool.tile([P, osh], mybir.dt.float32)
                nc.vector.tensor_tensor(
                    osb[:], psum[:], bias_sb[:], op=mybir.AluOpType.add)
                nc.scalar.dma_start(out[mt * P:(mt + 1) * P, :], osb[:])

    return xt, widx, lutx, bias, out


def make_core_inputs(input, lookup_table, weight, bias, *, m=M_FULL, k=D_IN,
                     osh=D_OUT // N_CORES, n_cores=N_CORES):
    """Host-side (value-preserving) sharding prep. Returns in_maps."""
    x2 = np.asarray(input, dtype=np.float32).reshape(m, k)
    xt = np.ascontiguousarray(x2.T)  # [k, m]
    lut_vals = np.asarray(lookup_table, dtype=np.float32).reshape(PALETTE)
    weight = np.asarray(weight)
    bias = np.asarray(bias, dtype=np.float32)

    # Expanded per-partition table: LUT values placed at slot (p%16)*256.
    lutx = np.zeros((P, 16 * PALETTE), dtype=np.float32)
    for p in range(P):
        s = p % 16
        lutx[p, s * PALETTE:(s + 1) * PALETTE] = lut_vals

    # ramp[p, s] = 256 * ((s*16 + p%16) // osh)
    p16 = (np.arange(P) % 16)[:, None]
    s_idx = np.arange(osh)[None, :]
    ramp = (PALETTE * ((s_idx * 16 + p16) // osh)).astype(np.int16)

    in_maps = []
    for c in range(n_cores):
        w_shard = weight[c * osh:(c + 1) * osh, :]  # [osh, k] int32
        wrapped = wrap_indices(np.ascontiguousarray(w_shard), k // P)
        in_maps.append({
            "xt": xt,
            "widx": wrapped.view(np.int16),
            "lutx": lutx,
            "ramp": ramp,
            "bias": bias[c * osh:(c + 1) * osh].reshape(1, osh),
        })
    return in_maps


def kernel(input, lookup_table, weight, bias, *, trace=False):
    osh = D_OUT // N_CORES
    nc = bacc.Bacc("TRN2", target_bir_lowering=False, debug=False,
                   num_devices=N_CORES)
    build_program(nc, m=M_FULL, k=D_IN, osh=osh)
    nc.compile()

    in_maps = make_core_inputs(input, lookup_table, weight, bias)
    res = run_bass_kernel_spmd(nc, in_maps, core_ids=list(range(N_CORES)),
                               trace=trace)
    out = np.concatenate([r["out"] for r in res.results], axis=1)
    out = np.ascontiguousarray(out.reshape(BATCH, SEQ, D_OUT), dtype=np.float32)
    if trace:
        kernel.last_results = res
    return out



# revision 2
# speedup vs baseline: 11.5786x; 11.5786x over previous
"""FP8-palettized linear kernel for 8x TRN2 NeuronCores (RLE dequant).

Computes: out[b,s,o] = sum_d input[b,s,d] * lookup_table[weight[o,d]] + bias[o]
with input [4,2048,4096] f32, weight [4096,4096] int32 (palette ids < 256),
lookup_table [256] f32, bias [4096] f32.

Strategy (column-parallel): each core owns 512 out-features; input
replicated. Host prep is index/layout only.

Dequant (per 128x512 W^T k-tile, exact, no random SBUF reads):
  lut diffs (f32, device) -> local_scatter lo/hi int16 halves to
  per-partition run-start slots -> DVE interleave to f32 stream [128,768]
  -> prefix-sum scan (runs become palette values) -> bf16 ->
  local_scatter back to true column positions = W^T tile bf16.
Host supplies the two index arrays from a per-(k-tile, partition)
counting sort of palette ids (layout transform of the weight input).

Matmul: X host-pretiled so each m-slab DMA is 128 rows x 16KB
contiguous; PSUM accumulation over 32 k-tiles per m-tile; DVE adds
bias; results DMA out.
"""

import contextlib

import numpy as np

import concourse.bacc as bacc
import concourse.mybir as mybir
import concourse.tile as tile
from concourse.bass_utils import run_bass_kernel_spmd

P = 128
N_CORES = 8

BATCH, SEQ, D_IN, D_OUT, PALETTE = 4, 2048, 4096, 4096, 256
M_FULL = BATCH * SEQ  # 8192
OSH = D_OUT // N_CORES  # 512
N_KT = D_IN // P  # 32
STREAM = OSH + PALETTE  # 768


def rle_host_prep(wT: np.ndarray):
    """wT: [k, osh] int32 palette ids. Pure index layout (counting sort).

    Returns sidx [128, n_kt*256] i16 (diff-mark slots) and
    widx2 [128, n_kt*768] i16 (stream slot -> column, or -1)."""
    k, osh = wT.shape
    n_kt = k // P
    V = wT.reshape(n_kt, P, osh)
    order = np.argsort(V, axis=-1, kind="stable")
    Vs = np.take_along_axis(V, order, axis=-1)

    base = (np.arange(n_kt * P) * PALETTE)[:, None]
    cnt = np.bincount((V.reshape(n_kt * P, osh) + base).ravel(),
                      minlength=n_kt * P * PALETTE).reshape(n_kt, P, PALETTE)
    cex = np.cumsum(cnt, axis=-1) - cnt
    sidx = (np.arange(PALETTE)[None, None, :] + cex).astype(np.int16)

    widx2 = np.full((n_kt, P, STREAM), -1, dtype=np.int16)
    slot = Vs + 1 + np.arange(osh)[None, None, :]
    np.put_along_axis(widx2, slot, order.astype(np.int16), axis=-1)

    sidx = np.ascontiguousarray(
        sidx.transpose(1, 0, 2).reshape(P, n_kt * PALETTE))
    widx2 = np.ascontiguousarray(
        widx2.transpose(1, 0, 2).reshape(P, n_kt * STREAM))
    return sidx, widx2


def build_program(nc, *, m=M_FULL, k=D_IN, osh=OSH, reps=1):
    n_kt = k // P
    n_mt = m // P

    xt4 = nc.dram_tensor("xt4", [m, k], mybir.dt.float32r,
                         kind="ExternalInput")
    lut_d = nc.dram_tensor("lut", [1, PALETTE], mybir.dt.float32,
                           kind="ExternalInput")
    sidx_d = nc.dram_tensor("sidx", [P, n_kt * PALETTE], mybir.dt.int16,
                            kind="ExternalInput")
    widx2_d = nc.dram_tensor("widx2", [P, n_kt * STREAM], mybir.dt.int16,
                             kind="ExternalInput")
    bias_d = nc.dram_tensor("bias", [1, osh], mybir.dt.float32,
                            kind="ExternalInput")
    out = nc.dram_tensor("out", [m, osh], mybir.dt.float32,
                         kind="ExternalOutput")

    with tile.TileContext(nc) as tc:
        with (
            tc.tile_pool(name="const", bufs=1) as const_pool,
            tc.tile_pool(name="idx", bufs=2) as idx_pool,
            tc.tile_pool(name="work", bufs=2) as work_pool,
            tc.tile_pool(name="wt", bufs=1) as wt_pool,
            tc.tile_pool(name="xs", bufs=3) as x_pool,
            tc.tile_pool(name="psum", bufs=2, space="PSUM") as psum_pool,
            tc.tile_pool(name="osb", bufs=2) as osb_pool,
            (tc.For_i(0, reps, 1) if reps > 1
             else contextlib.nullcontext()),
        ):
            # --- constants ---
            bias_row = const_pool.tile([1, osh], mybir.dt.float32, tag="brow")
            nc.sync.dma_start(bias_row[:], bias_d[:])
            bias_sb = const_pool.tile([P, osh], mybir.dt.float32, tag="bsb")
            nc.gpsimd.partition_broadcast(bias_sb[:], bias_row[:])

            lutpad = const_pool.tile([1, PALETTE + 1], mybir.dt.float32,
                                     tag="lutpad")
            nc.vector.memset(lutpad[:], 0.0)
            nc.sync.dma_start(lutpad[:, 1:PALETTE + 1], lut_d[:])
            dif1 = const_pool.tile([1, PALETTE], mybir.dt.float32, tag="dif1")
            nc.vector.tensor_tensor(dif1[:], lutpad[:, 1:PALETTE + 1],
                                    lutpad[:, 0:PALETTE],
                                    op=mybir.AluOpType.subtract)
            dif = const_pool.tile([P, PALETTE], mybir.dt.float32, tag="dif")
            nc.gpsimd.partition_broadcast(dif[:], dif1[:])
            dif_i16 = dif[:].bitcast(mybir.dt.int16)
            dlo = const_pool.tile([P, PALETTE], mybir.dt.int16, tag="dlo")
            dhi = const_pool.tile([P, PALETTE], mybir.dt.int16, tag="dhi")
            nc.vector.tensor_copy(
                dlo[:],
                dif_i16.rearrange("p (s two) -> p s two", two=2)[:, :, 0])
            nc.vector.tensor_copy(
                dhi[:],
                dif_i16.rearrange("p (s two) -> p s two", two=2)[:, :, 1])

            sidx_sb = const_pool.tile([P, n_kt * PALETTE], mybir.dt.int16,
                                      tag="sidx")
            nc.sync.dma_start(sidx_sb[:], sidx_d[:])

            # --- dequant: one W^T tile [128 d, osh o] bf16 per k-tile ---
            wt_tiles = []
            for kt in range(n_kt):
                w2 = idx_pool.tile([P, STREAM], mybir.dt.int16)
                nc.sync.dma_start(
                    w2[:], widx2_d[:, kt * STREAM:(kt + 1) * STREAM])

                bufA = work_pool.tile([P, STREAM], mybir.dt.int16)
                bufB = work_pool.tile([P, STREAM], mybir.dt.int16)
                si = sidx_sb[:, kt * PALETTE:(kt + 1) * PALETTE]
                nc.gpsimd.local_scatter(bufA[:], dlo[:], si, channels=P,
                                        num_elems=STREAM, num_idxs=PALETTE)
                nc.gpsimd.local_scatter(bufB[:], dhi[:], si, channels=P,
                                        num_elems=STREAM, num_idxs=PALETTE)

                C = work_pool.tile([P, STREAM], mybir.dt.float32)
                C16 = C[:].bitcast(mybir.dt.int16).rearrange(
                    "p (s two) -> p s two", two=2)
                nc.vector.tensor_copy(C16[:, :, 0], bufA[:])
                nc.vector.tensor_copy(C16[:, :, 1], bufB[:])

                Sbf = work_pool.tile([P, STREAM], mybir.dt.bfloat16)
                nc.vector.tensor_tensor_scan(
                    Sbf[:], C[:], C[:], 0.0,
                    op0=mybir.AluOpType.add, op1=mybir.AluOpType.bypass)

                wtb = work_pool.tile([P, osh], mybir.dt.bfloat16)
                nc.gpsimd.local_scatter(wtb[:], Sbf[:], w2[:], channels=P,
                                        num_elems=osh, num_idxs=STREAM)
                # f32r copy on the (otherwise idle) scalar engine: the PE
                # rejects mixed 32/16-bit operands, and X stays f32r.
                wt = wt_pool.tile([P, osh], mybir.dt.float32r,
                                  tag=f"wt{kt:02d}")
                nc.scalar.copy(wt[:], wtb[:])
                wt_tiles.append(wt)

            # --- matmul: out[m-tile, :] = X-slab @ W^T (+bias) ---
            for mt in range(n_mt):
                xslab = x_pool.tile([P, k], mybir.dt.float32r)
                nc.sync.dma_start(xslab[:], xt4[mt * P:(mt + 1) * P, :])
                psum = psum_pool.tile([P, osh], mybir.dt.float32)
                for kt in range(n_kt):
                    nc.tensor.matmul(
                        psum[:],
                        lhsT=xslab[:, kt * P:(kt + 1) * P],
                        rhs=wt_tiles[kt][:],
                        start=(kt == 0),
                        stop=(kt == n_kt - 1))
                osb = osb_pool.tile([P, osh], mybir.dt.float32)
                nc.vector.tensor_tensor(
                    osb[:], psum[:], bias_sb[:], op=mybir.AluOpType.add)
                nc.scalar.dma_start(out[mt * P:(mt + 1) * P, :], osb[:])

    return out


def pretile_x(input, m=M_FULL):
    """[B,S,D] f32 -> [m, 4096] where row (mt*128+p) holds, contiguously,
    X[mt*128+j, kt*128+p] for kt-major, j-minor. One m-slab DMA is then
    128 rows x 16KB contiguous."""
    x = np.asarray(input, dtype=np.float32).reshape(m, D_IN)
    x4 = x.reshape(m // P, P, N_KT, P)  # [mt, j, kt, p]
    xt4 = x4.transpose(0, 3, 2, 1)  # [mt, p, kt, j]
    return np.ascontiguousarray(xt4.reshape(m, D_IN))


def make_core_inputs(input, lookup_table, weight, bias):
    xt4 = pretile_x(input)
    lut = np.asarray(lookup_table, dtype=np.float32).reshape(1, PALETTE)
    weight = np.asarray(weight)
    bias = np.asarray(bias, dtype=np.float32)

    in_maps = []
    for c in range(N_CORES):
        w_shard = weight[c * OSH:(c + 1) * OSH, :]  # [osh, k] int32
        sidx, widx2 = rle_host_prep(np.ascontiguousarray(w_shard.T))
        in_maps.append({
            "xt4": xt4,
            "lut": lut,
            "sidx": sidx,
            "widx2": widx2,
            "bias": bias[c * OSH:(c + 1) * OSH].reshape(1, OSH),
        })
    return in_maps


def kernel(input, lookup_table, weight, bias, *, trace=False):
    nc = bacc.Bacc("TRN2", target_bir_lowering=False, debug=False,
                   num_devices=N_CORES)
    build_program(nc)
    nc.compile()

    in_maps = make_core_inputs(input, lookup_table, weight, bias)
    res = run_bass_kernel_spmd(nc, in_maps, core_ids=list(range(N_CORES)),
                               trace=trace)
    out = np.concatenate([r["out"] for r in res.results], axis=1)
    out = np.ascontiguousarray(out.reshape(BATCH, SEQ, D_OUT),
                               dtype=np.float32)
    if trace:
        kernel.last_results = res
    return out
